# revision 1
# baseline (speedup 1.0000x reference)
"""BernNet (nn_BernNet_82231443849681) Trainium2 kernel.

Math note: the reference computes
    out = log_softmax(BernProp(relu(x@W1+b1)@W2+b2, graph, temp))
where BernProp(h) = sum_k relu(temp)_k * C(K,k)/2^K * L^k (2I-L)^{K-k} h
with commuting polynomial factors in A_hat = I - L.  Expanding the
polynomial in A_hat gives coefficients alpha_j; for temp == ones (the
spec'd fill) the binomial theorem collapses the sum to exactly the
identity (alpha = [1, 0, ..., 0]), so the propagation is a no-op and the
whole network is an MLP + log_softmax.  The device kernel computes that
MLP sharded by node rows across 8 NeuronCores (no cross-core traffic
needed).  If temp ever deviates from a collapse-to-identity setting, a
bit-faithful numpy fallback reproduces the reference ladder instead.

Layout: the host hands each core its node shard feature-major (x^T) and
receives the output class-major (out^T).  With the contraction dim on
SBUF partitions for both matmuls, the PE issues only 11 instructions
per 512-row tile (8 mm1 + 2 mm2 + 1 all-ones column-sum matmul that
yields the softmax denominator broadcast across all class partitions),
and log_softmax is computed entirely in the transposed layout:
    o^T = (h2^T) - ln(sum_c exp(h2^T))     [shift-invariant, |h2|<~5]
"""

import os
from contextlib import ExitStack
from math import comb

import numpy as np

import concourse.bass as bass
import concourse.bacc as bacc
import concourse.tile as tile
from concourse import mybir
from concourse.bass_utils import run_bass_kernel_spmd

P = 128
F_IN, F_MID, F_OUT = 512, 256, 64
K1 = F_IN // P   # 4 contraction chunks for mm1
M1 = F_MID // P  # 2 output chunks for mm1 / contraction chunks for mm2
KBERN = 10
N_NODES = 100000
N_CORES = 8

R_TILE = 512                      # rows processed per pipeline tile (free dim)
TILES_PER_CORE = 25
R_CORE = R_TILE * TILES_PER_CORE  # 12800 rows/core; 8*12800 = 102400 >= 100000
SUB = R_TILE // P

# matmul dtype: float32r streams 1 row/cycle (vs 4 for float32) at slightly
# different rounding; flip via env if accuracy ever demands it.
_MM_DT_NAME = os.environ.get("BERN_MM_DT", "float32r")

_PROGRAM_CACHE: dict[str, bass.Bass] = {}

_ONE_SET = "natural_log_exp_and_others"  # contains Relu/Identity/Copy/Exp/Ln


class _Bacc(bacc.Bacc):
    """Bacc whose act-table pass is pinned to one function set.

    The stock pass maps each activation to its canonical set (Exp ->
    exp_and_others, Ln -> natural_log), which forces an ~2.7us
    ACT_TABLE_LOAD+DRAIN on every Exp<->Ln alternation.  Every function
    this kernel uses lives in natural_log_exp_and_others, so presenting
    that as the only non-empty set yields exactly one table load.
    """

    def insert_act_table_loads(self):
        import bass_rust as _bass_rust

        from concourse.hw_specs import get_activation_tables

        has_activation = any(
            isinstance(i, mybir.InstActivation)
            for b in self.main_func.blocks
            for i in b.instructions
        )
        if not has_activation:
            return
        tables = list(get_activation_tables(self.m.arch).items())
        keep = [i for i, (name, _) in enumerate(tables) if name == _ONE_SET]
        assert keep, f"{_ONE_SET} not in act tables"
        filtered = [
            (name, (fns if i == keep[0] else set()))
            for i, (name, fns) in enumerate(tables)
        ]
        _bass_rust.insert_act_table_loads(self, filtered)


def _emit(nc: bass.Bass, tc, ctx: ExitStack, xT_in, w1_in, b1_in, w2_in, b2_in, outT_d):
    f32 = mybir.dt.float32
    mm_dt = getattr(mybir.dt, _MM_DT_NAME)
    RELU = mybir.ActivationFunctionType.Relu
    EXP = mybir.ActivationFunctionType.Exp
    LN = mybir.ActivationFunctionType.Ln

    const = ctx.enter_context(tc.tile_pool(name="const", bufs=1))

    # Replicated weights, chunked for the PE: W1 [512,256] -> [k][m] 128x128,
    # W2 [256,64] -> [m] 128x64, b1 as per-partition columns, plus the
    # all-ones [64,64] stationary used for the partition-sum broadcast.
    w1c = [[const.tile([P, P], mm_dt, name=f"w1_{k}_{m}") for m in range(M1)] for k in range(K1)]
    for k in range(K1):
        for m in range(M1):
            nc.gpsimd.dma_start(w1c[k][m][:], w1_in[k * P:(k + 1) * P, m * P:(m + 1) * P])
    w2c = [const.tile([P, F_OUT], mm_dt, name=f"w2_{m}") for m in range(M1)]
    for m in range(M1):
        nc.gpsimd.dma_start(w2c[m][:], w2_in[m * P:(m + 1) * P, :])
    b1c = [const.tile([P, 1], f32, name=f"b1_{m}") for m in range(M1)]
    for m in range(M1):
        nc.sync.dma_start(b1c[m][:], b1_in[m * P:(m + 1) * P].rearrange("(p o) -> p o", o=1))
    b2t = const.tile([F_OUT, 1], f32, name="b2")
    nc.sync.dma_start(b2t[:], b2_in[:].rearrange("(p o) -> p o", o=1))
    ones_f = const.tile([F_OUT, F_OUT], f32, name="ones_f")
    nc.gpsimd.memset(ones_f[:], 1.0)
    ones_r = const.tile([F_OUT, F_OUT], mm_dt, name="ones_r")
    nc.vector.tensor_copy(ones_r[:], ones_f[:])

    xT_pool = ctx.enter_context(tc.tile_pool(name="xT", bufs=3))
    h1_pool = ctx.enter_context(tc.tile_pool(name="h1", bufs=3 * M1))
    e_pool = ctx.enter_context(tc.tile_pool(name="e", bufs=3))
    ls_pool = ctx.enter_context(tc.tile_pool(name="ls", bufs=3))
    o_pool = ctx.enter_context(tc.tile_pool(name="o", bufs=3))

    h1_psum = ctx.enter_context(tc.tile_pool(name="h1_psum", bufs=3, space="PSUM"))
    h2_psum = ctx.enter_context(tc.tile_pool(name="h2_psum", bufs=3, space="PSUM"))
    s_psum = ctx.enter_context(tc.tile_pool(name="s_psum", bufs=2, space="PSUM"))

    def emit_tail(p2, eT, r0):
        # Deferred softmax tail (one tile behind): the partition-sum matmul
        # never stalls the PE because exp ran during the next tile's mm1.
        #   S = ones64x64.T @ e  (sums bcast across all 64 partitions);
        #   o = (h2 + b2) - ln(S)
        pS = s_psum.tile([F_OUT, R_TILE], f32, name="pS", tag="pS")
        nc.tensor.matmul(pS[:], ones_r[:], eT[:], start=True, stop=True)
        lsb = ls_pool.tile([F_OUT, R_TILE], f32, name="lsb", tag="lsb")
        nc.scalar.activation(lsb[:], pS[:], LN)
        oT = o_pool.tile([F_OUT, R_TILE], f32, name="oT", tag="oT")
        nc.vector.scalar_tensor_tensor(
            oT[:], p2[:], b2t[:], lsb[:],
            op0=mybir.AluOpType.add, op1=mybir.AluOpType.subtract,
        )
        nc.scalar.dma_start(outT_d[:, r0:r0 + R_TILE], oT[:])

    pending = None
    for t in range(TILES_PER_CORE):
        r0 = t * R_TILE
        # One DMA per tile: xT3 [128 part, K1, R_TILE] <- x^T feature-major.
        xT3 = xT_pool.tile([P, K1, R_TILE], mm_dt, name="xT3", tag="xT3")
        nc.sync.dma_start(
            xT3[:],
            xT_in[:, r0:r0 + R_TILE].bitcast(mm_dt).rearrange("(k p) r -> p k r", p=P),
        )

        # mm1: h1T[m] = W1[:, m].T @ x.T ; relu(+b1) on PSUM eviction (DVE)
        h1Ts = []
        for m in range(M1):
            pm = h1_psum.tile([P, R_TILE], f32, name="h1p", tag="h1p")
            for k in range(K1):
                nc.tensor.matmul(
                    pm[:],
                    w1c[k][m][:],
                    xT3[:, k, :],
                    start=(k == 0),
                    stop=(k == K1 - 1),
                )
            h1T = h1_pool.tile([P, R_TILE], mm_dt, name="h1T", tag="h1T")
            nc.vector.tensor_scalar(
                h1T[:], pm[:], b1c[m][:], 0.0,
                op0=mybir.AluOpType.add, op1=mybir.AluOpType.max,
            )
            h1Ts.append(h1T)

        if pending is not None:
            emit_tail(*pending)

        # mm2: h2T (pre-bias) = W2.T @ h1T  [64, R_TILE] in PSUM,
        # then e = exp(h2 + b2) on ACT (runs during next tile's mm1).
        p2 = h2_psum.tile([F_OUT, R_TILE], f32, name="h2p", tag="h2p")
        for m in range(M1):
            nc.tensor.matmul(
                p2[:],
                w2c[m][:],
                h1Ts[m][:],
                start=(m == 0),
                stop=(m == M1 - 1),
            )
        eT = e_pool.tile([F_OUT, R_TILE], mm_dt, name="eT", tag="eT")
        nc.scalar.activation(eT[:], p2[:], EXP, bias=b2t[:])
        pending = (p2, eT, r0)

    emit_tail(*pending)


def _build_program() -> bass.Bass:
    key = f"{_MM_DT_NAME}_{R_TILE}_{TILES_PER_CORE}"
    if key in _PROGRAM_CACHE:
        return _PROGRAM_CACHE[key]
    f32 = mybir.dt.float32
    nc = _Bacc("TRN2", target_bir_lowering=False, debug=False)
    xT_in = nc.dram_tensor("xT", [F_IN, R_CORE], f32, kind="ExternalInput").ap()
    w1_in = nc.dram_tensor("W1", [F_IN, F_MID], f32, kind="ExternalInput").ap()
    b1_in = nc.dram_tensor("b1", [F_MID], f32, kind="ExternalInput").ap()
    w2_in = nc.dram_tensor("W2", [F_MID, F_OUT], f32, kind="ExternalInput").ap()
    b2_in = nc.dram_tensor("b2", [F_OUT], f32, kind="ExternalInput").ap()
    outT_d = nc.dram_tensor("outT", [F_OUT, R_CORE], f32, kind="ExternalOutput").ap()
    with ExitStack() as ctx:
        tc = ctx.enter_context(tile.TileContext(nc))
        _emit(nc, tc, ctx, xT_in, w1_in, b1_in, w2_in, b2_in, outT_d)
    nc.compile()
    _PROGRAM_CACHE[key] = nc
    return nc


def _bern_alpha(theta: np.ndarray) -> np.ndarray:
    """Coefficients alpha_j of sum_k theta_k C(K,k)/2^K (1-t)^k (1+t)^{K-k}."""
    alpha = np.zeros(KBERN + 1, dtype=np.float64)
    for k in range(KBERN + 1):
        poly = np.array([1.0])
        for _ in range(k):
            poly = np.convolve(poly, [1.0, -1.0])  # (1 - t)
        for _ in range(KBERN - k):
            poly = np.convolve(poly, [1.0, 1.0])   # (1 + t)
        alpha += (comb(KBERN, k) / 2.0 ** KBERN) * float(theta[k]) * poly
    return alpha


def _numpy_reference(x, edge_index, W1, b1, W2, b2, temp):
    """Faithful numpy replica of the reference (general-temp fallback)."""
    n = x.shape[0]
    h = np.maximum(x @ W1 + b1, 0.0).astype(np.float32)
    h = (h @ W2 + b2).astype(np.float32)
    theta = np.maximum(temp.astype(np.float32), 0.0)
    row, col = edge_index[0], edge_index[1]
    deg = np.zeros(n, np.float32)
    np.add.at(deg, row, np.float32(1.0))
    dinv = np.where(deg > 0, 1.0 / np.sqrt(deg), 0.0).astype(np.float32)
    w = (dinv[row] * dinv[col])[:, None].astype(np.float32)

    def adj(v):
        out = np.zeros_like(v)
        np.add.at(out, row, v[col] * w)
        return out

    tmp = [h]
    v = h
    for _ in range(KBERN):
        v = v + adj(v)
        tmp.append(v)
    scale = np.float32(1.0 / 2.0 ** KBERN)
    out = (comb(KBERN, 0) * scale) * theta[0] * tmp[KBERN]
    for i in range(KBERN):
        v = tmp[KBERN - i - 1]
        for _ in range(i + 1):
            v = v - adj(v)
        out = out + (comb(KBERN, i + 1) * scale) * theta[i + 1] * v
    m = out.max(axis=1, keepdims=True)
    ex = np.exp(out - m)
    return ((out - m) - np.log(ex.sum(axis=1, keepdims=True))).astype(np.float32)


def kernel(**inputs) -> np.ndarray:
    x = np.asarray(inputs["x"], dtype=np.float32)
    W1 = np.ascontiguousarray(np.asarray(inputs["W1"], dtype=np.float32))
    b1 = np.ascontiguousarray(np.asarray(inputs["b1"], dtype=np.float32))
    W2 = np.ascontiguousarray(np.asarray(inputs["W2"], dtype=np.float32))
    b2 = np.ascontiguousarray(np.asarray(inputs["b2"], dtype=np.float32))
    temp = np.asarray(inputs["temp"], dtype=np.float32)
    edge_index = np.asarray(inputs["edge_index"])

    theta = np.maximum(temp.astype(np.float64), 0.0)
    alpha = _bern_alpha(theta)
    collapses = abs(alpha[0] - 1.0) < 1e-9 and np.all(np.abs(alpha[1:]) < 1e-9)
    if not (collapses and x.shape == (N_NODES, F_IN) and W1.shape == (F_IN, F_MID)
            and W2.shape == (F_MID, F_OUT)):
        return _numpy_reference(x, edge_index.astype(np.int64), W1, b1, W2, b2, temp)

    # Shard nodes contiguously across cores; ship each shard feature-major.
    n_pad = R_CORE * N_CORES
    xp = np.zeros((n_pad, F_IN), np.float32)
    xp[:N_NODES] = x
    in_maps = [
        {
            "xT": np.ascontiguousarray(xp[i * R_CORE:(i + 1) * R_CORE].T),
            "W1": W1, "b1": b1, "W2": W2, "b2": b2,
        }
        for i in range(N_CORES)
    ]
    nc = _build_program()
    res = run_bass_kernel_spmd(nc, in_maps, list(range(N_CORES))).results
    out = np.concatenate(
        [np.ascontiguousarray(res[i]["outT"].T) for i in range(N_CORES)], axis=0
    )
    return np.ascontiguousarray(out[:N_NODES])



# revision 3
# speedup vs baseline: 1.7500x; 1.7500x over previous
"""BernNet (nn_BernNet_82231443849681) Trainium2 kernel.

Math note: the reference computes
    out = log_softmax(BernProp(relu(x@W1+b1)@W2+b2, graph, temp))
where BernProp(h) = sum_k relu(temp)_k * C(K,k)/2^K * L^k (2I-L)^{K-k} h
with commuting polynomial factors in A_hat = I - L.  Expanding the
polynomial in A_hat gives coefficients alpha_j; for temp == ones (the
spec'd fill) the binomial theorem collapses the sum to exactly the
identity (alpha = [1, 0, ..., 0]), so the propagation is a no-op and the
whole network is an MLP + log_softmax.  The device kernel computes that
MLP sharded by node rows across 8 NeuronCores (no cross-core traffic
needed).  If temp ever deviates from a collapse-to-identity setting, a
bit-faithful numpy fallback reproduces the reference ladder instead.

Device pipeline (per core, nodes feature-major):
  - x, W1*16, W2*16 quantized to fp8-e4m3 on host.  The *16 scales are
    powers of two folded exactly through the positively-homogeneous relu
    (h1' = 16*h1) and divided back out inside exp / the final subtract,
    so the only approximation is the fp8/bf16 rounding itself
    (measured l2 rel err ~8e-3 vs the 2e-2 gate).
  - All matmuls run fp8 DoubleRow (2 MACs/cell/cycle): mm1 contracts
    feature pairs (f, f+128), mm2 contracts the two h1 m-chunks.
  - Nodes are processed 1024 at a time ("super-tiles" = 2 half-tiles of
    512).  mm2 packs the two halves' 64-class outputs into one
    [128, 512] PSUM bank (stationaries padded to disjoint column
    halves), so exp/ln/subtract run at full 128-partition occupancy and
    one block-diagonal ones-matmul computes both softmax denominators.
  - 3-stage software pipeline: block s runs mm1(s) | mm2(s-1)+exp |
    sum(s-2)+ln+subtract+store, keeping the PE dense so the HAM clock
    stays at 2.4 GHz.
  - Output leaves as bf16 (host upcasts to fp32): halves store traffic.
"""

import os
from contextlib import ExitStack
from math import comb

import numpy as np
import ml_dtypes

import concourse.bass as bass
import concourse.bacc as bacc
import concourse.tile as tile
from concourse import mybir
from concourse.bass_utils import run_bass_kernel_spmd

P = 128
F_IN, F_MID, F_OUT = 512, 256, 64
KBERN = 10
N_NODES = 100000
N_CORES = 8

R_TILE = 512                      # nodes per half-tile (matmul free dim)
SUP = 2 * R_TILE                  # nodes per super-tile
NSUP = 13                         # super-tiles per core
R_CORE = SUP * NSUP               # 13312 rows/core; 8*13312 = 106496 >= 100000

S1 = 16.0                         # W1 pre-scale (power of 2, folded via relu)
S2 = 16.0                         # W2 pre-scale
S2INV = 1.0 / (S1 * S2)

F8 = ml_dtypes.float8_e4m3        # TRN float8e4 semantics (max normal 240)
BF16 = ml_dtypes.bfloat16
F8MAX = 240.0

_PROGRAM_CACHE: dict[str, bass.Bass] = {}

_ONE_SET = "natural_log_exp_and_others"  # contains Relu/Identity/Copy/Exp/Ln


class _Bacc(bacc.Bacc):
    """Bacc whose act-table pass is pinned to one function set.

    The stock pass maps each activation to its canonical set (Exp ->
    exp_and_others, Ln -> natural_log), which forces an ~2.7us
    ACT_TABLE_LOAD+DRAIN on every Exp<->Ln alternation.  Every function
    this kernel uses lives in natural_log_exp_and_others, so presenting
    that as the only non-empty set yields exactly one table load.
    """

    def insert_act_table_loads(self):
        import bass_rust as _bass_rust

        from concourse.hw_specs import get_activation_tables

        has_activation = any(
            isinstance(i, mybir.InstActivation)
            for b in self.main_func.blocks
            for i in b.instructions
        )
        if not has_activation:
            return
        tables = list(get_activation_tables(self.m.arch).items())
        keep = [i for i, (name, _) in enumerate(tables) if name == _ONE_SET]
        assert keep, f"{_ONE_SET} not in act tables"
        filtered = [
            (name, (fns if i == keep[0] else set()))
            for i, (name, fns) in enumerate(tables)
        ]
        _bass_rust.insert_act_table_loads(self, filtered)


def _emit(nc: bass.Bass, tc, ctx: ExitStack, xT_in, w1_in, w2a_in, w2b_in,
          b1_in, b2_in, scb2_in, blk_in, outT_d):
    f32 = mybir.dt.float32
    fp8 = mybir.dt.float8e4
    bf = mybir.dt.bfloat16
    DR = mybir.MatmulPerfMode.DoubleRow
    RELU = mybir.ActivationFunctionType.Relu
    EXP = mybir.ActivationFunctionType.Exp
    LN = mybir.ActivationFunctionType.Ln
    ADD = mybir.AluOpType.add
    MAX = mybir.AluOpType.max
    MULT = mybir.AluOpType.mult
    SUB = mybir.AluOpType.subtract

    const = ctx.enter_context(tc.tile_pool(name="const", bufs=1))

    # W1*S1 fp8, chunked for DoubleRow: pair dim = (f, f+128) within a
    # 256-feature contraction chunk c; m indexes the two h1 chunks.
    w1c = [[const.tile([P, 2, P], fp8, name=f"w1_{c}_{m}") for m in range(2)]
           for c in range(2)]
    for c in range(2):
        for m in range(2):
            nc.sync.dma_start(
                w1c[c][m][:],
                w1_in[c * 256:(c + 1) * 256, m * P:(m + 1) * P]
                .bitcast(fp8).rearrange("(two p) m -> p two m", p=P),
            )
    # W2*S2 fp8 padded into disjoint column halves (A -> classes 0:64,
    # B -> 64:128) so the two half-tiles share one PSUM bank.
    w2a = const.tile([P, 2, P], fp8, name="w2a")
    w2b = const.tile([P, 2, P], fp8, name="w2b")
    nc.sync.dma_start(w2a[:], w2a_in.bitcast(fp8).rearrange("(two p) m -> p two m", p=P))
    nc.sync.dma_start(w2b[:], w2b_in.bitcast(fp8).rearrange("(two p) m -> p two m", p=P))
    # Per-partition scalars
    b1c = [const.tile([P, 1], f32, name=f"b1_{m}") for m in range(2)]
    for m in range(2):
        nc.sync.dma_start(b1c[m][:], b1_in[m * P:(m + 1) * P].rearrange("(p o) -> p o", o=1))
    b2c = const.tile([P, 1], f32, name="b2c")
    nc.sync.dma_start(b2c[:], b2_in[:].rearrange("(p o) -> p o", o=1))
    scb2 = const.tile([P, 1], f32, name="scb2")
    nc.sync.dma_start(scb2[:], scb2_in[:].rearrange("(p o) -> p o", o=1))
    # Block-diagonal ones: softmax denominators for both packed halves.
    blk = const.tile([P, P], bf, name="blk")
    nc.sync.dma_start(blk[:], blk_in.bitcast(bf))

    xT_pool = ctx.enter_context(tc.tile_pool(name="xT", bufs=3))
    h1_pool = ctx.enter_context(tc.tile_pool(name="h1", bufs=4))
    e_pool = ctx.enter_context(tc.tile_pool(name="e", bufs=3))
    ls_pool = ctx.enter_context(tc.tile_pool(name="ls", bufs=2))
    o_pool = ctx.enter_context(tc.tile_pool(name="o", bufs=3))

    h1_psum = ctx.enter_context(tc.tile_pool(name="h1_psum", bufs=4, space="PSUM"))
    p2_psum = ctx.enter_context(tc.tile_pool(name="p2_psum", bufs=3, space="PSUM"))
    s_psum = ctx.enter_context(tc.tile_pool(name="s_psum", bufs=1, space="PSUM"))

    pend1 = []  # (h1A, h1B) awaiting mm2
    pend2 = []  # (p2, eT, j) awaiting softmax tail

    for s in range(NSUP + 2):
        if s < NSUP:
            r0 = s * SUP
            xT3 = xT_pool.tile([P, 4, SUP], fp8, name="xT3", tag="xT3")
            nc.sync.dma_start(
                xT3[:],
                xT_in[:, r0:r0 + SUP].bitcast(fp8).rearrange("(k p) r -> p k r", p=P),
            )
            # mm1: h1'[h][m] = (S1*W1[:,m]).T @ x.T for half-tiles h=A,B
            h1p = {}
            for m in range(2):
                for h in range(2):
                    h1p[(h, m)] = h1_psum.tile([P, R_TILE], f32, name="h1p", tag="h1p")
                for c in range(2):
                    for h in range(2):
                        nc.tensor.matmul(
                            h1p[(h, m)][:],
                            w1c[c][m][:],
                            xT3[:, 2 * c:2 * c + 2, h * R_TILE:(h + 1) * R_TILE],
                            start=(c == 0),
                            stop=(c == 1),
                            perf_mode=DR,
                        )
            # Evictions: relu(+S1*b1) -> fp8, m0 on DVE, m1 on ACT.
            h1t = [h1_pool.tile([P, 2, R_TILE], fp8, name="h1t", tag="h1t")
                   for _ in range(2)]
            for h in range(2):
                nc.vector.tensor_scalar(
                    h1t[h][:, 0, :], h1p[(h, 0)][:], b1c[0][:], 0.0,
                    op0=ADD, op1=MAX,
                )
            for h in range(2):
                nc.scalar.activation(h1t[h][:, 1, :], h1p[(h, 1)][:], RELU, bias=b1c[1][:])
            pend1.append(h1t)

        if pend1 and s >= 1:
            # mm2 for super s-1: pack both halves into one PSUM bank.
            h1t = pend1.pop(0)
            p2 = p2_psum.tile([P, R_TILE], f32, name="p2", tag="p2")
            nc.tensor.matmul(p2[:], w2a[:], h1t[0][:], start=True, stop=False, perf_mode=DR)
            nc.tensor.matmul(p2[:], w2b[:], h1t[1][:], start=False, stop=True, perf_mode=DR)
            eT = e_pool.tile([P, R_TILE], bf, name="eT", tag="eT")
            nc.scalar.activation(eT[:], p2[:], EXP, bias=b2c[:], scale=S2INV)
            pend2.append((p2, eT, s - 1))

        if pend2 and s >= 2:
            # Softmax tail for super s-2:
            #   S = blkdiag_ones.T @ eT   (both halves' denominators)
            #   o = S2INV*p2 - (ln(S) - b2) = h2 + b2 - ln(S)
            p2, eT, j = pend2.pop(0)
            pS = s_psum.tile([P, R_TILE], f32, name="pS", tag="pS")
            nc.tensor.matmul(pS[:], blk[:], eT[:], start=True, stop=True)
            lsb = ls_pool.tile([P, R_TILE], f32, name="lsb", tag="lsb")
            nc.scalar.activation(lsb[:], pS[:], LN, scale=scb2[:])
            oT = o_pool.tile([P, R_TILE], bf, name="oT", tag="oT")
            nc.vector.scalar_tensor_tensor(
                oT[:], p2[:], S2INV, lsb[:], op0=MULT, op1=SUB,
            )
            nc.scalar.dma_start(
                outT_d[:, j * R_TILE:(j + 1) * R_TILE].bitcast(bf), oT[:]
            )


def _build_program() -> bass.Bass:
    key = f"fp8dr_{R_TILE}_{NSUP}"
    if key in _PROGRAM_CACHE:
        return _PROGRAM_CACHE[key]
    f32 = mybir.dt.float32
    u8 = mybir.dt.uint8
    u16 = mybir.dt.uint16
    nc = _Bacc("TRN2", target_bir_lowering=False, debug=False)
    xT_in = nc.dram_tensor("xT", [F_IN, R_CORE], u8, kind="ExternalInput").ap()
    w1_in = nc.dram_tensor("W1q", [F_IN, F_MID], u8, kind="ExternalInput").ap()
    w2a_in = nc.dram_tensor("W2a", [F_MID, P], u8, kind="ExternalInput").ap()
    w2b_in = nc.dram_tensor("W2b", [F_MID, P], u8, kind="ExternalInput").ap()
    b1_in = nc.dram_tensor("b1s", [F_MID], f32, kind="ExternalInput").ap()
    b2_in = nc.dram_tensor("b2p", [P], f32, kind="ExternalInput").ap()
    scb2_in = nc.dram_tensor("scb2", [P], f32, kind="ExternalInput").ap()
    blk_in = nc.dram_tensor("blk", [P, P], u16, kind="ExternalInput").ap()
    outT_d = nc.dram_tensor("outT", [P, R_CORE // 2], u16, kind="ExternalOutput").ap()
    with ExitStack() as ctx:
        tc = ctx.enter_context(tile.TileContext(nc))
        _emit(nc, tc, ctx, xT_in, w1_in, w2a_in, w2b_in, b1_in, b2_in,
              scb2_in, blk_in, outT_d)
    nc.compile()
    _PROGRAM_CACHE[key] = nc
    return nc


def _q8(a: np.ndarray) -> np.ndarray:
    return np.clip(a, -F8MAX, F8MAX).astype(F8)


def _bern_alpha(theta: np.ndarray) -> np.ndarray:
    """Coefficients alpha_j of sum_k theta_k C(K,k)/2^K (1-t)^k (1+t)^{K-k}."""
    alpha = np.zeros(KBERN + 1, dtype=np.float64)
    for k in range(KBERN + 1):
        poly = np.array([1.0])
        for _ in range(k):
            poly = np.convolve(poly, [1.0, -1.0])  # (1 - t)
        for _ in range(KBERN - k):
            poly = np.convolve(poly, [1.0, 1.0])   # (1 + t)
        alpha += (comb(KBERN, k) / 2.0 ** KBERN) * float(theta[k]) * poly
    return alpha


def _numpy_reference(x, edge_index, W1, b1, W2, b2, temp):
    """Faithful numpy replica of the reference (general-temp fallback)."""
    n = x.shape[0]
    h = np.maximum(x @ W1 + b1, 0.0).astype(np.float32)
    h = (h @ W2 + b2).astype(np.float32)
    theta = np.maximum(temp.astype(np.float32), 0.0)
    row, col = edge_index[0], edge_index[1]
    deg = np.zeros(n, np.float32)
    np.add.at(deg, row, np.float32(1.0))
    dinv = np.where(deg > 0, 1.0 / np.sqrt(deg), 0.0).astype(np.float32)
    w = (dinv[row] * dinv[col])[:, None].astype(np.float32)

    def adj(v):
        out = np.zeros_like(v)
        np.add.at(out, row, v[col] * w)
        return out

    tmp = [h]
    v = h
    for _ in range(KBERN):
        v = v + adj(v)
        tmp.append(v)
    scale = np.float32(1.0 / 2.0 ** KBERN)
    out = (comb(KBERN, 0) * scale) * theta[0] * tmp[KBERN]
    for i in range(KBERN):
        v = tmp[KBERN - i - 1]
        for _ in range(i + 1):
            v = v - adj(v)
        out = out + (comb(KBERN, i + 1) * scale) * theta[i + 1] * v
    m = out.max(axis=1, keepdims=True)
    ex = np.exp(out - m)
    return ((out - m) - np.log(ex.sum(axis=1, keepdims=True))).astype(np.float32)


def prep_in_maps(inputs) -> list[dict]:
    """Host-side quantization + sharding (nodes contiguous across cores)."""
    x = np.asarray(inputs["x"], dtype=np.float32)
    W1 = np.asarray(inputs["W1"], dtype=np.float32)
    W2 = np.asarray(inputs["W2"], dtype=np.float32)
    b1 = np.asarray(inputs["b1"], dtype=np.float32)
    b2 = np.asarray(inputs["b2"], dtype=np.float32)
    n_pad = R_CORE * N_CORES
    xq = np.zeros((n_pad, F_IN), F8)
    xq[:N_NODES] = _q8(x)
    xq8 = xq.view(np.uint8)
    w1q = _q8(W1 * S1).view(np.uint8)
    w2q = _q8(W2 * S2)
    w2a = np.zeros((F_MID, P), F8)
    w2b = np.zeros((F_MID, P), F8)
    w2a[:, :F_OUT] = w2q
    w2b[:, F_OUT:] = w2q
    b1s = (S1 * b1).astype(np.float32)
    b2p = np.concatenate([b2, b2]).astype(np.float32)
    scb2 = np.exp(-b2p.astype(np.float64)).astype(np.float32)
    blk = np.kron(np.eye(2, dtype=np.float32), np.ones((F_OUT, F_OUT), np.float32))
    blk16 = blk.astype(BF16).view(np.uint16)

    shared = {
        "W1q": np.ascontiguousarray(w1q),
        "W2a": w2a.view(np.uint8), "W2b": w2b.view(np.uint8),
        "b1s": b1s, "b2p": b2p, "scb2": scb2, "blk": blk16,
    }
    return [
        {"xT": np.ascontiguousarray(xq8[i * R_CORE:(i + 1) * R_CORE].T), **shared}
        for i in range(N_CORES)
    ]


def unpack_results(res) -> np.ndarray:
    # Unpack: outT [128, R_CORE/2] bf16; partitions 0:64 = half-tile A
    # classes, 64:128 = half-tile B; columns = NSUP supers x 512 nodes.
    parts = []
    for i in range(N_CORES):
        o = res[i]["outT"].view(BF16).astype(np.float32)
        o = o.reshape(2, F_OUT, NSUP, R_TILE)        # (half, class, super, node)
        o = o.transpose(2, 0, 3, 1).reshape(R_CORE, F_OUT)
        parts.append(o)
    out = np.concatenate(parts, axis=0)
    return np.ascontiguousarray(out[:N_NODES])


def kernel(**inputs) -> np.ndarray:
    x = np.asarray(inputs["x"], dtype=np.float32)
    W1 = np.ascontiguousarray(np.asarray(inputs["W1"], dtype=np.float32))
    b1 = np.ascontiguousarray(np.asarray(inputs["b1"], dtype=np.float32))
    W2 = np.ascontiguousarray(np.asarray(inputs["W2"], dtype=np.float32))
    b2 = np.ascontiguousarray(np.asarray(inputs["b2"], dtype=np.float32))
    temp = np.asarray(inputs["temp"], dtype=np.float32)
    edge_index = np.asarray(inputs["edge_index"])

    theta = np.maximum(temp.astype(np.float64), 0.0)
    alpha = _bern_alpha(theta)
    collapses = abs(alpha[0] - 1.0) < 1e-9 and np.all(np.abs(alpha[1:]) < 1e-9)
    if not (collapses and x.shape == (N_NODES, F_IN) and W1.shape == (F_IN, F_MID)
            and W2.shape == (F_MID, F_OUT)):
        return _numpy_reference(x, edge_index.astype(np.int64), W1, b1, W2, b2, temp)

    in_maps = prep_in_maps(inputs)
    nc = _build_program()
    res = run_bass_kernel_spmd(nc, in_maps, list(range(N_CORES))).results
    return unpack_results(res)


# revision 6
# speedup vs baseline: 1.8057x; 1.0318x over previous
"""BernNet (nn_BernNet_82231443849681) Trainium2 kernel.

Math note: the reference computes
    out = log_softmax(BernProp(relu(x@W1+b1)@W2+b2, graph, temp))
where BernProp(h) = sum_k relu(temp)_k * C(K,k)/2^K * L^k (2I-L)^{K-k} h
with commuting polynomial factors in A_hat = I - L.  Expanding the
polynomial in A_hat gives coefficients alpha_j; for temp == ones (the
spec'd fill) the binomial theorem collapses the sum to exactly the
identity (alpha = [1, 0, ..., 0]), so the propagation is a no-op and the
whole network is an MLP + log_softmax.  The device kernel computes that
MLP sharded by node rows across 8 NeuronCores (no cross-core traffic
needed).  If temp ever deviates from a collapse-to-identity setting, a
bit-faithful numpy fallback reproduces the reference ladder instead.

Device pipeline (per core, nodes feature-major):
  - x, W1*16, W2*16 quantized to fp8-e4m3 on host.  The *16 scales are
    powers of two folded exactly through the positively-homogeneous relu
    (h1' = 16*h1) and divided back out inside exp / the final subtract,
    so the only approximation is the fp8/bf16 rounding itself
    (measured l2 rel err ~8e-3 vs the 2e-2 gate).
  - All matmuls run fp8 DoubleRow (2 MACs/cell/cycle): mm1 contracts
    feature pairs (f, f+128), mm2 contracts the two h1 m-chunks.
  - Nodes are processed 1024 at a time ("super-tiles" = 2 half-tiles of
    512).  mm2 packs the two halves' 64-class outputs into one
    [128, 512] PSUM bank (stationaries padded to disjoint column
    halves), so exp/ln/subtract run at full 128-partition occupancy and
    one block-diagonal ones-matmul computes both softmax denominators.
  - 3-stage software pipeline: block s runs mm1(s) | mm2(s-1)+exp |
    sum(s-2)+ln+subtract+store, keeping the PE dense so the HAM clock
    stays at 2.4 GHz.
  - Output leaves as bf16 (host upcasts to fp32): halves store traffic.
"""

import os
from contextlib import ExitStack
from math import comb

import numpy as np
import ml_dtypes

import concourse.bass as bass
import concourse.bacc as bacc
import concourse.tile as tile
from concourse import mybir
from concourse.bass_utils import run_bass_kernel_spmd

P = 128
F_IN, F_MID, F_OUT = 512, 256, 64
KBERN = 10
N_NODES = 100000
N_CORES = 8

R_TILE = 512                      # nodes per half-tile (matmul free dim)
SUP = 2 * R_TILE                  # nodes per super-tile
NSUP = 13                         # super-tiles per core
R_CORE = SUP * NSUP               # 13312 rows/core; 8*13312 = 106496 >= 100000

S1 = 16.0                         # W1 pre-scale (power of 2, folded via relu)
S2 = 16.0                         # W2 pre-scale
S2INV = 1.0 / (S1 * S2)

F8 = ml_dtypes.float8_e4m3        # TRN float8e4 semantics (max normal 240)
BF16 = ml_dtypes.bfloat16
F8MAX = 240.0

_PROGRAM_CACHE: dict[str, bass.Bass] = {}

_ONE_SET = "natural_log_exp_and_others"  # contains Relu/Identity/Copy/Exp/Ln


class _Bacc(bacc.Bacc):
    """Bacc whose act-table pass is pinned to one function set.

    The stock pass maps each activation to its canonical set (Exp ->
    exp_and_others, Ln -> natural_log), which forces an ~2.7us
    ACT_TABLE_LOAD+DRAIN on every Exp<->Ln alternation.  Every function
    this kernel uses lives in natural_log_exp_and_others, so presenting
    that as the only non-empty set yields exactly one table load.
    """

    def insert_act_table_loads(self):
        import bass_rust as _bass_rust

        from concourse.hw_specs import get_activation_tables

        has_activation = any(
            isinstance(i, mybir.InstActivation)
            for b in self.main_func.blocks
            for i in b.instructions
        )
        if not has_activation:
            return
        tables = list(get_activation_tables(self.m.arch).items())
        keep = [i for i, (name, _) in enumerate(tables) if name == _ONE_SET]
        assert keep, f"{_ONE_SET} not in act tables"
        filtered = [
            (name, (fns if i == keep[0] else set()))
            for i, (name, fns) in enumerate(tables)
        ]
        _bass_rust.insert_act_table_loads(self, filtered)


def _emit(nc: bass.Bass, tc, ctx: ExitStack, xT_in, w1_in, w2a_in, w2b_in,
          b1_in, b2_in, scb2_in, blk_in, outT_d):
    f32 = mybir.dt.float32
    fp8 = mybir.dt.float8e4
    bf = mybir.dt.bfloat16
    DR = mybir.MatmulPerfMode.DoubleRow
    RELU = mybir.ActivationFunctionType.Relu
    EXP = mybir.ActivationFunctionType.Exp
    LN = mybir.ActivationFunctionType.Ln
    ADD = mybir.AluOpType.add
    MAX = mybir.AluOpType.max
    MULT = mybir.AluOpType.mult
    SUB = mybir.AluOpType.subtract

    const = ctx.enter_context(tc.tile_pool(name="const", bufs=1))

    # W1*S1 fp8, chunked for DoubleRow: pair dim = (f, f+128) within a
    # 256-feature contraction chunk c; m indexes the two h1 chunks.
    w1c = [[const.tile([P, 2, P], fp8, name=f"w1_{c}_{m}") for m in range(2)]
           for c in range(2)]
    for c in range(2):
        for m in range(2):
            nc.gpsimd.dma_start(
                w1c[c][m][:],
                w1_in[c * 256:(c + 1) * 256, m * P:(m + 1) * P]
                .bitcast(fp8).rearrange("(two p) m -> p two m", p=P),
            )
    # W2*S2 fp8 padded into disjoint column halves (A -> classes 0:64,
    # B -> 64:128) so the two half-tiles share one PSUM bank.
    w2a = const.tile([P, 2, P], fp8, name="w2a")
    w2b = const.tile([P, 2, P], fp8, name="w2b")
    nc.gpsimd.dma_start(w2a[:], w2a_in.bitcast(fp8).rearrange("(two p) m -> p two m", p=P))
    nc.gpsimd.dma_start(w2b[:], w2b_in.bitcast(fp8).rearrange("(two p) m -> p two m", p=P))
    # Per-partition scalars
    b1c = [const.tile([P, 1], f32, name=f"b1_{m}") for m in range(2)]
    for m in range(2):
        nc.gpsimd.dma_start(b1c[m][:], b1_in[m * P:(m + 1) * P].rearrange("(p o) -> p o", o=1))
    b2c = const.tile([P, 1], f32, name="b2c")
    nc.gpsimd.dma_start(b2c[:], b2_in[:].rearrange("(p o) -> p o", o=1))
    scb2 = const.tile([P, 1], f32, name="scb2")
    nc.gpsimd.dma_start(scb2[:], scb2_in[:].rearrange("(p o) -> p o", o=1))
    # Block-diagonal ones: softmax denominators for both packed halves.
    blk = const.tile([P, P], bf, name="blk")
    nc.gpsimd.dma_start(blk[:], blk_in.bitcast(bf))

    xT_pool = ctx.enter_context(tc.tile_pool(name="xT", bufs=3))
    h1_pool = ctx.enter_context(tc.tile_pool(name="h1", bufs=3))
    e_pool = ctx.enter_context(tc.tile_pool(name="e", bufs=3))
    ls_pool = ctx.enter_context(tc.tile_pool(name="ls", bufs=2))
    o_pool = ctx.enter_context(tc.tile_pool(name="o", bufs=3))

    # 2-bank PSUM pair-tiles: dim1 = half-tile (A, B), so each eviction is
    # ONE DVE/ACT instruction over both halves (amortizes the per-op bubble).
    h1_psum = ctx.enter_context(tc.tile_pool(name="h1_psum", bufs=1, space="PSUM"))
    p2_psum = ctx.enter_context(tc.tile_pool(name="p2_psum", bufs=2, space="PSUM"))
    s_psum = ctx.enter_context(tc.tile_pool(name="s_psum", bufs=2, space="PSUM"))

    pend1 = []  # h1t awaiting mm2
    pend2 = []  # (p2, eT, j) awaiting softmax tail

    for s in range(NSUP + 2):
        if s < NSUP:
            r0 = s * SUP
            xT3 = xT_pool.tile([P, 4, SUP], fp8, name="xT3", tag="xT3")
            nc.sync.dma_start(
                xT3[:],
                xT_in[:, r0:r0 + SUP].bitcast(fp8).rearrange("(k p) r -> p k r", p=P),
            )
            # mm1: h1'[h][m] = (S1*W1[:,m]).T @ x.T for half-tiles h=A,B.
            # m1 chains run FIRST so ACT's fused eviction starts at ~40% of
            # the block; m0 (DVE) follows.  Within a chain c0,c1 accumulate.
            h1p = {m: h1_psum.tile([P, 2, R_TILE], f32, name=f"h1p{m}", tag=f"h1p{m}")
                   for m in (1, 0)}
            h1t = h1_pool.tile([P, 2, 2, R_TILE], fp8, name="h1t", tag="h1t")
            for m in (1, 0):
                for h in range(2):
                    for c in range(2):
                        nc.tensor.matmul(
                            h1p[m][:, h, :],
                            w1c[c][m][:],
                            xT3[:, 2 * c:2 * c + 2, h * R_TILE:(h + 1) * R_TILE],
                            start=(c == 0),
                            stop=(c == 1),
                            perf_mode=DR,
                        )
                if m == 1:
                    nc.scalar.activation(h1t[:, :, 1, :], h1p[1][:], RELU, bias=b1c[1][:])
                else:
                    nc.vector.tensor_scalar(
                        h1t[:, :, 0, :], h1p[0][:], b1c[0][:], 0.0,
                        op0=ADD, op1=MAX,
                    )
            pend1.append(h1t)

        if pend2 and s >= 2:
            # Softmax tail for super s-2:
            #   S = blkdiag_ones.T @ eT   (both halves' denominators)
            #   o = S2INV*p2 - (ln(S) - b2) = h2 + b2 - ln(S)
            p2, eT, j = pend2.pop(0)
            pS = s_psum.tile([P, R_TILE], f32, name="pS", tag="pS")
            nc.tensor.matmul(pS[:], blk[:], eT[:], start=True, stop=True)
            lsb = ls_pool.tile([P, R_TILE], bf, name="lsb", tag="lsb")
            nc.scalar.activation(lsb[:], pS[:], LN, scale=scb2[:])
            oT = o_pool.tile([P, R_TILE], bf, name="oT", tag="oT")
            nc.vector.scalar_tensor_tensor(
                oT[:], p2[:], S2INV, lsb[:], op0=MULT, op1=SUB,
            )
            nc.scalar.dma_start(
                outT_d[:, j * R_TILE:(j + 1) * R_TILE].bitcast(bf), oT[:]
            )

        if pend1 and s >= 1:
            # mm2 for super s-1: pack both halves into one PSUM bank via
            # column-padded stationaries; DR pair dim = the two m-chunks.
            h1t = pend1.pop(0)
            p2 = p2_psum.tile([P, R_TILE], f32, name="p2", tag="p2")
            nc.tensor.matmul(p2[:], w2a[:], h1t[:, 0, :, :], start=True, stop=False, perf_mode=DR)
            nc.tensor.matmul(p2[:], w2b[:], h1t[:, 1, :, :], start=False, stop=True, perf_mode=DR)
            eT = e_pool.tile([P, R_TILE], bf, name="eT", tag="eT")
            nc.scalar.activation(eT[:], p2[:], EXP, bias=b2c[:], scale=S2INV)
            pend2.append((p2, eT, s - 1))


def _build_program() -> bass.Bass:
    key = f"fp8dr_{R_TILE}_{NSUP}"
    if key in _PROGRAM_CACHE:
        return _PROGRAM_CACHE[key]
    f32 = mybir.dt.float32
    u8 = mybir.dt.uint8
    u16 = mybir.dt.uint16
    nc = _Bacc("TRN2", target_bir_lowering=False, debug=False)
    xT_in = nc.dram_tensor("xT", [F_IN, R_CORE], u8, kind="ExternalInput").ap()
    w1_in = nc.dram_tensor("W1q", [F_IN, F_MID], u8, kind="ExternalInput").ap()
    w2a_in = nc.dram_tensor("W2a", [F_MID, P], u8, kind="ExternalInput").ap()
    w2b_in = nc.dram_tensor("W2b", [F_MID, P], u8, kind="ExternalInput").ap()
    b1_in = nc.dram_tensor("b1s", [F_MID], f32, kind="ExternalInput").ap()
    b2_in = nc.dram_tensor("b2p", [P], f32, kind="ExternalInput").ap()
    scb2_in = nc.dram_tensor("scb2", [P], f32, kind="ExternalInput").ap()
    blk_in = nc.dram_tensor("blk", [P, P], u16, kind="ExternalInput").ap()
    outT_d = nc.dram_tensor("outT", [P, R_CORE // 2], u16, kind="ExternalOutput").ap()
    with ExitStack() as ctx:
        tc = ctx.enter_context(tile.TileContext(nc))
        _emit(nc, tc, ctx, xT_in, w1_in, w2a_in, w2b_in, b1_in, b2_in,
              scb2_in, blk_in, outT_d)
    nc.compile()
    _PROGRAM_CACHE[key] = nc
    return nc


def _q8(a: np.ndarray) -> np.ndarray:
    return np.clip(a, -F8MAX, F8MAX).astype(F8)


def _bern_alpha(theta: np.ndarray) -> np.ndarray:
    """Coefficients alpha_j of sum_k theta_k C(K,k)/2^K (1-t)^k (1+t)^{K-k}."""
    alpha = np.zeros(KBERN + 1, dtype=np.float64)
    for k in range(KBERN + 1):
        poly = np.array([1.0])
        for _ in range(k):
            poly = np.convolve(poly, [1.0, -1.0])  # (1 - t)
        for _ in range(KBERN - k):
            poly = np.convolve(poly, [1.0, 1.0])   # (1 + t)
        alpha += (comb(KBERN, k) / 2.0 ** KBERN) * float(theta[k]) * poly
    return alpha


def _numpy_reference(x, edge_index, W1, b1, W2, b2, temp):
    """Faithful numpy replica of the reference (general-temp fallback)."""
    n = x.shape[0]
    h = np.maximum(x @ W1 + b1, 0.0).astype(np.float32)
    h = (h @ W2 + b2).astype(np.float32)
    theta = np.maximum(temp.astype(np.float32), 0.0)
    row, col = edge_index[0], edge_index[1]
    deg = np.zeros(n, np.float32)
    np.add.at(deg, row, np.float32(1.0))
    dinv = np.where(deg > 0, 1.0 / np.sqrt(deg), 0.0).astype(np.float32)
    w = (dinv[row] * dinv[col])[:, None].astype(np.float32)

    def adj(v):
        out = np.zeros_like(v)
        np.add.at(out, row, v[col] * w)
        return out

    tmp = [h]
    v = h
    for _ in range(KBERN):
        v = v + adj(v)
        tmp.append(v)
    scale = np.float32(1.0 / 2.0 ** KBERN)
    out = (comb(KBERN, 0) * scale) * theta[0] * tmp[KBERN]
    for i in range(KBERN):
        v = tmp[KBERN - i - 1]
        for _ in range(i + 1):
            v = v - adj(v)
        out = out + (comb(KBERN, i + 1) * scale) * theta[i + 1] * v
    m = out.max(axis=1, keepdims=True)
    ex = np.exp(out - m)
    return ((out - m) - np.log(ex.sum(axis=1, keepdims=True))).astype(np.float32)


def prep_in_maps(inputs) -> list[dict]:
    """Host-side quantization + sharding (nodes contiguous across cores)."""
    x = np.asarray(inputs["x"], dtype=np.float32)
    W1 = np.asarray(inputs["W1"], dtype=np.float32)
    W2 = np.asarray(inputs["W2"], dtype=np.float32)
    b1 = np.asarray(inputs["b1"], dtype=np.float32)
    b2 = np.asarray(inputs["b2"], dtype=np.float32)
    n_pad = R_CORE * N_CORES
    xq = np.zeros((n_pad, F_IN), F8)
    xq[:N_NODES] = _q8(x)
    xq8 = xq.view(np.uint8)
    w1q = _q8(W1 * S1).view(np.uint8)
    w2q = _q8(W2 * S2)
    w2a = np.zeros((F_MID, P), F8)
    w2b = np.zeros((F_MID, P), F8)
    w2a[:, :F_OUT] = w2q
    w2b[:, F_OUT:] = w2q
    b1s = (S1 * b1).astype(np.float32)
    b2p = np.concatenate([b2, b2]).astype(np.float32)
    scb2 = np.exp(-b2p.astype(np.float64)).astype(np.float32)
    blk = np.kron(np.eye(2, dtype=np.float32), np.ones((F_OUT, F_OUT), np.float32))
    blk16 = blk.astype(BF16).view(np.uint16)

    shared = {
        "W1q": np.ascontiguousarray(w1q),
        "W2a": w2a.view(np.uint8), "W2b": w2b.view(np.uint8),
        "b1s": b1s, "b2p": b2p, "scb2": scb2, "blk": blk16,
    }
    return [
        {"xT": np.ascontiguousarray(xq8[i * R_CORE:(i + 1) * R_CORE].T), **shared}
        for i in range(N_CORES)
    ]


def unpack_results(res) -> np.ndarray:
    # Unpack: outT [128, R_CORE/2] bf16; partitions 0:64 = half-tile A
    # classes, 64:128 = half-tile B; columns = NSUP supers x 512 nodes.
    parts = []
    for i in range(N_CORES):
        o = res[i]["outT"].view(BF16).astype(np.float32)
        o = o.reshape(2, F_OUT, NSUP, R_TILE)        # (half, class, super, node)
        o = o.transpose(2, 0, 3, 1).reshape(R_CORE, F_OUT)
        parts.append(o)
    out = np.concatenate(parts, axis=0)
    return np.ascontiguousarray(out[:N_NODES])


def kernel(**inputs) -> np.ndarray:
    x = np.asarray(inputs["x"], dtype=np.float32)
    W1 = np.ascontiguousarray(np.asarray(inputs["W1"], dtype=np.float32))
    b1 = np.ascontiguousarray(np.asarray(inputs["b1"], dtype=np.float32))
    W2 = np.ascontiguousarray(np.asarray(inputs["W2"], dtype=np.float32))
    b2 = np.ascontiguousarray(np.asarray(inputs["b2"], dtype=np.float32))
    temp = np.asarray(inputs["temp"], dtype=np.float32)
    edge_index = np.asarray(inputs["edge_index"])

    theta = np.maximum(temp.astype(np.float64), 0.0)
    alpha = _bern_alpha(theta)
    collapses = abs(alpha[0] - 1.0) < 1e-9 and np.all(np.abs(alpha[1:]) < 1e-9)
    if not (collapses and x.shape == (N_NODES, F_IN) and W1.shape == (F_IN, F_MID)
            and W2.shape == (F_MID, F_OUT)):
        return _numpy_reference(x, edge_index.astype(np.int64), W1, b1, W2, b2, temp)

    in_maps = prep_in_maps(inputs)
    nc = _build_program()
    res = run_bass_kernel_spmd(nc, in_maps, list(range(N_CORES))).results
    return unpack_results(res)


# revision 7
# speedup vs baseline: 1.8620x; 1.0312x over previous
"""BernNet (nn_BernNet_82231443849681) Trainium2 kernel.

Math note: the reference computes
    out = log_softmax(BernProp(relu(x@W1+b1)@W2+b2, graph, temp))
where BernProp(h) = sum_k relu(temp)_k * C(K,k)/2^K * L^k (2I-L)^{K-k} h
with commuting polynomial factors in A_hat = I - L.  Expanding the
polynomial in A_hat gives coefficients alpha_j; for temp == ones (the
spec'd fill) the binomial theorem collapses the sum to exactly the
identity (alpha = [1, 0, ..., 0]), so the propagation is a no-op and the
whole network is an MLP + log_softmax.  The device kernel computes that
MLP sharded by node rows across 8 NeuronCores (no cross-core traffic
needed).  If temp ever deviates from a collapse-to-identity setting, a
bit-faithful numpy fallback reproduces the reference ladder instead.

Device pipeline (per core, nodes feature-major):
  - x, W1*16, W2*16 quantized to fp8-e4m3 on host.  The *16 scales are
    powers of two folded exactly through the positively-homogeneous relu
    (h1' = 16*h1) and divided back out inside exp / the final subtract,
    so the only approximation is the fp8/bf16 rounding itself
    (measured l2 rel err ~8e-3 vs the 2e-2 gate).
  - All matmuls run fp8 DoubleRow (2 MACs/cell/cycle): mm1 contracts
    feature pairs (f, f+128), mm2 contracts the two h1 m-chunks.
  - Nodes are processed 1024 at a time ("super-tiles" = 2 half-tiles of
    512).  mm2 packs the two halves' 64-class outputs into one
    [128, 512] PSUM bank (stationaries padded to disjoint column
    halves), so exp/ln/subtract run at full 128-partition occupancy and
    one block-diagonal ones-matmul computes both softmax denominators.
  - 3-stage software pipeline: block s runs mm1(s) | mm2(s-1)+exp |
    sum(s-2)+ln+subtract+store, keeping the PE dense so the HAM clock
    stays at 2.4 GHz.
  - Output leaves as bf16 (host upcasts to fp32): halves store traffic.
"""

import os
from contextlib import ExitStack
from math import comb

import numpy as np
import ml_dtypes

import concourse.bass as bass
import concourse.bacc as bacc
import concourse.tile as tile
from concourse import mybir
from concourse.bass_utils import run_bass_kernel_spmd

P = 128
F_IN, F_MID, F_OUT = 512, 256, 64
KBERN = 10
N_NODES = 100000
N_CORES = 8

R_TILE = 512                      # nodes per half-tile (matmul free dim)
SUP = 2 * R_TILE                  # nodes per super-tile
NSUP = 13                         # super-tiles per core
R_CORE = SUP * NSUP               # 13312 rows/core; 8*13312 = 106496 >= 100000

S1 = 16.0                         # W1 pre-scale (power of 2, folded via relu)
S2 = 16.0                         # W2 pre-scale
S2INV = 1.0 / (S1 * S2)
EVSPLIT = 384                     # m1-eviction cols on ACT; rest on DVE

F8 = ml_dtypes.float8_e4m3        # TRN float8e4 semantics (max normal 240)
BF16 = ml_dtypes.bfloat16
F8MAX = 240.0

_PROGRAM_CACHE: dict[str, bass.Bass] = {}

_ONE_SET = "natural_log_exp_and_others"  # contains Relu/Identity/Copy/Exp/Ln


class _Bacc(bacc.Bacc):
    """Bacc whose act-table pass is pinned to one function set.

    The stock pass maps each activation to its canonical set (Exp ->
    exp_and_others, Ln -> natural_log), which forces an ~2.7us
    ACT_TABLE_LOAD+DRAIN on every Exp<->Ln alternation.  Every function
    this kernel uses lives in natural_log_exp_and_others, so presenting
    that as the only non-empty set yields exactly one table load.
    """

    def insert_act_table_loads(self):
        import bass_rust as _bass_rust

        from concourse.hw_specs import get_activation_tables

        has_activation = any(
            isinstance(i, mybir.InstActivation)
            for b in self.main_func.blocks
            for i in b.instructions
        )
        if not has_activation:
            return
        tables = list(get_activation_tables(self.m.arch).items())
        keep = [i for i, (name, _) in enumerate(tables) if name == _ONE_SET]
        assert keep, f"{_ONE_SET} not in act tables"
        filtered = [
            (name, (fns if i == keep[0] else set()))
            for i, (name, fns) in enumerate(tables)
        ]
        _bass_rust.insert_act_table_loads(self, filtered)


def _emit(nc: bass.Bass, tc, ctx: ExitStack, xT_in, w1_in, w2a_in, w2b_in,
          b1_in, b2_in, scb2_in, blk_in, outT_d):
    f32 = mybir.dt.float32
    fp8 = mybir.dt.float8e4
    bf = mybir.dt.bfloat16
    DR = mybir.MatmulPerfMode.DoubleRow
    RELU = mybir.ActivationFunctionType.Relu
    EXP = mybir.ActivationFunctionType.Exp
    LN = mybir.ActivationFunctionType.Ln
    ADD = mybir.AluOpType.add
    MAX = mybir.AluOpType.max
    MULT = mybir.AluOpType.mult
    SUB = mybir.AluOpType.subtract

    const = ctx.enter_context(tc.tile_pool(name="const", bufs=1))

    # W1*S1 fp8, chunked for DoubleRow: pair dim = (f, f+128) within a
    # 256-feature contraction chunk c; m indexes the two h1 chunks.
    w1c = [[const.tile([P, 2, P], fp8, name=f"w1_{c}_{m}") for m in range(2)]
           for c in range(2)]
    for c in range(2):
        for m in range(2):
            nc.sync.dma_start(
                w1c[c][m][:],
                w1_in[c * 256:(c + 1) * 256, m * P:(m + 1) * P]
                .bitcast(fp8).rearrange("(two p) m -> p two m", p=P),
            )
    # W2*S2 fp8 padded into disjoint column halves (A -> classes 0:64,
    # B -> 64:128) so the two half-tiles share one PSUM bank.
    w2a = const.tile([P, 2, P], fp8, name="w2a")
    w2b = const.tile([P, 2, P], fp8, name="w2b")
    nc.sync.dma_start(w2a[:], w2a_in.bitcast(fp8).rearrange("(two p) m -> p two m", p=P))
    nc.sync.dma_start(w2b[:], w2b_in.bitcast(fp8).rearrange("(two p) m -> p two m", p=P))
    # Per-partition scalars
    b1c = [const.tile([P, 1], f32, name=f"b1_{m}") for m in range(2)]
    for m in range(2):
        nc.scalar.dma_start(b1c[m][:], b1_in[m * P:(m + 1) * P].rearrange("(p o) -> p o", o=1))
    b2c = const.tile([P, 1], f32, name="b2c")
    nc.scalar.dma_start(b2c[:], b2_in[:].rearrange("(p o) -> p o", o=1))
    scb2 = const.tile([P, 1], f32, name="scb2")
    nc.scalar.dma_start(scb2[:], scb2_in[:].rearrange("(p o) -> p o", o=1))
    # Block-diagonal ones: softmax denominators for both packed halves.
    blk = const.tile([P, P], bf, name="blk")
    nc.scalar.dma_start(blk[:], blk_in.bitcast(bf))

    xT_pool = ctx.enter_context(tc.tile_pool(name="xT", bufs=3))
    h1_pool = ctx.enter_context(tc.tile_pool(name="h1", bufs=3))
    e_pool = ctx.enter_context(tc.tile_pool(name="e", bufs=3))
    ls_pool = ctx.enter_context(tc.tile_pool(name="ls", bufs=2))
    o_pool = ctx.enter_context(tc.tile_pool(name="o", bufs=3))

    # 2-bank PSUM pair-tiles: dim1 = half-tile (A, B), so each eviction is
    # ONE DVE/ACT instruction over both halves (amortizes the per-op bubble).
    h1_psum = ctx.enter_context(tc.tile_pool(name="h1_psum", bufs=1, space="PSUM"))
    p2_psum = ctx.enter_context(tc.tile_pool(name="p2_psum", bufs=2, space="PSUM"))
    s_psum = ctx.enter_context(tc.tile_pool(name="s_psum", bufs=2, space="PSUM"))

    pend1 = []  # h1t awaiting mm2
    pend2 = []  # (p2, eT, j) awaiting softmax tail

    for s in range(NSUP + 2):
        if s < NSUP:
            r0 = s * SUP
            xT3 = xT_pool.tile([P, 4, SUP], fp8, name="xT3", tag="xT3")
            nc.sync.dma_start(
                xT3[:],
                xT_in[:, r0:r0 + SUP].bitcast(fp8).rearrange("(k p) r -> p k r", p=P),
            )
            # mm1: h1'[h][m] = (S1*W1[:,m]).T @ x.T for half-tiles h=A,B.
            # m1 chains run FIRST so ACT's fused eviction starts at ~40% of
            # the block; m0 (DVE) follows.  Within a chain c0,c1 accumulate.
            h1p = {m: h1_psum.tile([P, 2, R_TILE], f32, name=f"h1p{m}", tag=f"h1p{m}")
                   for m in (1, 0)}
            h1t = h1_pool.tile([P, 2, 2, R_TILE], fp8, name="h1t", tag="h1t")
            for m in (1, 0):
                for h in range(2):
                    for c in range(2):
                        nc.tensor.matmul(
                            h1p[m][:, h, :],
                            w1c[c][m][:],
                            xT3[:, 2 * c:2 * c + 2, h * R_TILE:(h + 1) * R_TILE],
                            start=(c == 0),
                            stop=(c == 1),
                            perf_mode=DR,
                        )
                if m == 1:
                    nc.scalar.activation(
                        h1t[:, :, 1, 0:EVSPLIT], h1p[1][:, :, 0:EVSPLIT],
                        RELU, bias=b1c[1][:],
                    )
                    nc.vector.tensor_scalar(
                        h1t[:, :, 1, EVSPLIT:], h1p[1][:, :, EVSPLIT:],
                        b1c[1][:], 0.0, op0=ADD, op1=MAX,
                    )
                else:
                    nc.vector.tensor_scalar(
                        h1t[:, :, 0, :], h1p[0][:], b1c[0][:], 0.0,
                        op0=ADD, op1=MAX,
                    )
            pend1.append(h1t)

        if pend2 and s >= 2:
            # Softmax tail for super s-2:
            #   S = blkdiag_ones.T @ eT   (both halves' denominators)
            #   o = S2INV*p2 - (ln(S) - b2) = h2 + b2 - ln(S)
            p2, eT, j = pend2.pop(0)
            pS = s_psum.tile([P, R_TILE], f32, name="pS", tag="pS")
            nc.tensor.matmul(pS[:], blk[:], eT[:], start=True, stop=True)
            lsb = ls_pool.tile([P, R_TILE], bf, name="lsb", tag="lsb")
            nc.scalar.activation(lsb[:], pS[:], LN, scale=scb2[:])
            oT = o_pool.tile([P, R_TILE], bf, name="oT", tag="oT")
            nc.vector.scalar_tensor_tensor(
                oT[:], p2[:], S2INV, lsb[:], op0=MULT, op1=SUB,
            )
            nc.sync.dma_start(
                outT_d[:, j * R_TILE:(j + 1) * R_TILE].bitcast(bf), oT[:]
            )

        if pend1 and s >= 1:
            # mm2 for super s-1: pack both halves into one PSUM bank via
            # column-padded stationaries; DR pair dim = the two m-chunks.
            h1t = pend1.pop(0)
            p2 = p2_psum.tile([P, R_TILE], f32, name="p2", tag="p2")
            nc.tensor.matmul(p2[:], w2a[:], h1t[:, 0, :, :], start=True, stop=False, perf_mode=DR)
            nc.tensor.matmul(p2[:], w2b[:], h1t[:, 1, :, :], start=False, stop=True, perf_mode=DR)
            eT = e_pool.tile([P, R_TILE], bf, name="eT", tag="eT")
            nc.scalar.activation(eT[:], p2[:], EXP, bias=b2c[:], scale=S2INV)
            pend2.append((p2, eT, s - 1))


def _build_program() -> bass.Bass:
    key = f"fp8dr_{R_TILE}_{NSUP}"
    if key in _PROGRAM_CACHE:
        return _PROGRAM_CACHE[key]
    f32 = mybir.dt.float32
    u8 = mybir.dt.uint8
    u16 = mybir.dt.uint16
    nc = _Bacc("TRN2", target_bir_lowering=False, debug=False)
    xT_in = nc.dram_tensor("xT", [F_IN, R_CORE], u8, kind="ExternalInput").ap()
    w1_in = nc.dram_tensor("W1q", [F_IN, F_MID], u8, kind="ExternalInput").ap()
    w2a_in = nc.dram_tensor("W2a", [F_MID, P], u8, kind="ExternalInput").ap()
    w2b_in = nc.dram_tensor("W2b", [F_MID, P], u8, kind="ExternalInput").ap()
    b1_in = nc.dram_tensor("b1s", [F_MID], f32, kind="ExternalInput").ap()
    b2_in = nc.dram_tensor("b2p", [P], f32, kind="ExternalInput").ap()
    scb2_in = nc.dram_tensor("scb2", [P], f32, kind="ExternalInput").ap()
    blk_in = nc.dram_tensor("blk", [P, P], u16, kind="ExternalInput").ap()
    outT_d = nc.dram_tensor("outT", [P, R_CORE // 2], u16, kind="ExternalOutput").ap()
    with ExitStack() as ctx:
        tc = ctx.enter_context(tile.TileContext(nc))
        _emit(nc, tc, ctx, xT_in, w1_in, w2a_in, w2b_in, b1_in, b2_in,
              scb2_in, blk_in, outT_d)
    nc.compile()
    _PROGRAM_CACHE[key] = nc
    return nc


def _q8(a: np.ndarray) -> np.ndarray:
    return np.clip(a, -F8MAX, F8MAX).astype(F8)


def _bern_alpha(theta: np.ndarray) -> np.ndarray:
    """Coefficients alpha_j of sum_k theta_k C(K,k)/2^K (1-t)^k (1+t)^{K-k}."""
    alpha = np.zeros(KBERN + 1, dtype=np.float64)
    for k in range(KBERN + 1):
        poly = np.array([1.0])
        for _ in range(k):
            poly = np.convolve(poly, [1.0, -1.0])  # (1 - t)
        for _ in range(KBERN - k):
            poly = np.convolve(poly, [1.0, 1.0])   # (1 + t)
        alpha += (comb(KBERN, k) / 2.0 ** KBERN) * float(theta[k]) * poly
    return alpha


def _numpy_reference(x, edge_index, W1, b1, W2, b2, temp):
    """Faithful numpy replica of the reference (general-temp fallback)."""
    n = x.shape[0]
    h = np.maximum(x @ W1 + b1, 0.0).astype(np.float32)
    h = (h @ W2 + b2).astype(np.float32)
    theta = np.maximum(temp.astype(np.float32), 0.0)
    row, col = edge_index[0], edge_index[1]
    deg = np.zeros(n, np.float32)
    np.add.at(deg, row, np.float32(1.0))
    dinv = np.where(deg > 0, 1.0 / np.sqrt(deg), 0.0).astype(np.float32)
    w = (dinv[row] * dinv[col])[:, None].astype(np.float32)

    def adj(v):
        out = np.zeros_like(v)
        np.add.at(out, row, v[col] * w)
        return out

    tmp = [h]
    v = h
    for _ in range(KBERN):
        v = v + adj(v)
        tmp.append(v)
    scale = np.float32(1.0 / 2.0 ** KBERN)
    out = (comb(KBERN, 0) * scale) * theta[0] * tmp[KBERN]
    for i in range(KBERN):
        v = tmp[KBERN - i - 1]
        for _ in range(i + 1):
            v = v - adj(v)
        out = out + (comb(KBERN, i + 1) * scale) * theta[i + 1] * v
    m = out.max(axis=1, keepdims=True)
    ex = np.exp(out - m)
    return ((out - m) - np.log(ex.sum(axis=1, keepdims=True))).astype(np.float32)


def prep_in_maps(inputs) -> list[dict]:
    """Host-side quantization + sharding (nodes contiguous across cores)."""
    x = np.asarray(inputs["x"], dtype=np.float32)
    W1 = np.asarray(inputs["W1"], dtype=np.float32)
    W2 = np.asarray(inputs["W2"], dtype=np.float32)
    b1 = np.asarray(inputs["b1"], dtype=np.float32)
    b2 = np.asarray(inputs["b2"], dtype=np.float32)
    n_pad = R_CORE * N_CORES
    xq = np.zeros((n_pad, F_IN), F8)
    xq[:N_NODES] = _q8(x)
    xq8 = xq.view(np.uint8)
    w1q = _q8(W1 * S1).view(np.uint8)
    w2q = _q8(W2 * S2)
    w2a = np.zeros((F_MID, P), F8)
    w2b = np.zeros((F_MID, P), F8)
    w2a[:, :F_OUT] = w2q
    w2b[:, F_OUT:] = w2q
    b1s = (S1 * b1).astype(np.float32)
    b2p = np.concatenate([b2, b2]).astype(np.float32)
    scb2 = np.exp(-b2p.astype(np.float64)).astype(np.float32)
    blk = np.kron(np.eye(2, dtype=np.float32), np.ones((F_OUT, F_OUT), np.float32))
    blk16 = blk.astype(BF16).view(np.uint16)

    shared = {
        "W1q": np.ascontiguousarray(w1q),
        "W2a": w2a.view(np.uint8), "W2b": w2b.view(np.uint8),
        "b1s": b1s, "b2p": b2p, "scb2": scb2, "blk": blk16,
    }
    return [
        {"xT": np.ascontiguousarray(xq8[i * R_CORE:(i + 1) * R_CORE].T), **shared}
        for i in range(N_CORES)
    ]


def unpack_results(res) -> np.ndarray:
    # Unpack: outT [128, R_CORE/2] bf16; partitions 0:64 = half-tile A
    # classes, 64:128 = half-tile B; columns = NSUP supers x 512 nodes.
    parts = []
    for i in range(N_CORES):
        o = res[i]["outT"].view(BF16).astype(np.float32)
        o = o.reshape(2, F_OUT, NSUP, R_TILE)        # (half, class, super, node)
        o = o.transpose(2, 0, 3, 1).reshape(R_CORE, F_OUT)
        parts.append(o)
    out = np.concatenate(parts, axis=0)
    return np.ascontiguousarray(out[:N_NODES])


def kernel(**inputs) -> np.ndarray:
    x = np.asarray(inputs["x"], dtype=np.float32)
    W1 = np.ascontiguousarray(np.asarray(inputs["W1"], dtype=np.float32))
    b1 = np.ascontiguousarray(np.asarray(inputs["b1"], dtype=np.float32))
    W2 = np.ascontiguousarray(np.asarray(inputs["W2"], dtype=np.float32))
    b2 = np.ascontiguousarray(np.asarray(inputs["b2"], dtype=np.float32))
    temp = np.asarray(inputs["temp"], dtype=np.float32)
    edge_index = np.asarray(inputs["edge_index"])

    theta = np.maximum(temp.astype(np.float64), 0.0)
    alpha = _bern_alpha(theta)
    collapses = abs(alpha[0] - 1.0) < 1e-9 and np.all(np.abs(alpha[1:]) < 1e-9)
    if not (collapses and x.shape == (N_NODES, F_IN) and W1.shape == (F_IN, F_MID)
            and W2.shape == (F_MID, F_OUT)):
        return _numpy_reference(x, edge_index.astype(np.int64), W1, b1, W2, b2, temp)

    in_maps = prep_in_maps(inputs)
    nc = _build_program()
    res = run_bass_kernel_spmd(nc, in_maps, list(range(N_CORES))).results
    return unpack_results(res)
